# revision 2
# baseline (speedup 1.0000x reference)
"""Trainium2 Bass kernel for B-spline curve evaluation (nn_CurveEval), v2.

Per-span cubic-coefficient formulation:
  For each curve and knot span i (i=3..99), expand the cubic curve piece in
  the local variable x = u - U[i]:  C_d(x) = K0 + K1 x + K2 x^2 + K3 x^3.
  The 12 K coefficients (4 coeffs x 3 dims) are built per span on cheap
  [128,97] tiles (Cox-de Boor done symbolically; signs folded into
  subtract ops).  Then only 13 staircases (12 K + the left knot U[s]) are
  materialized over the 512 eval points via local_scatter + fill-forward
  scan, and the curve is evaluated with a 6-op Horner per dim.

Engine split per tile: DVE runs stage A (span search), the small recips /
level-2 algebra, the 13 scans and the Horner; Pool (gpsimd) runs the
level-3 algebra, the 66-op control-point combine (standard library) and
the 14 local_scatters (scatter library; library reloads are ~10ns and are
re-inserted post-Tile at transitions since the scheduler hoists them).
Curves with near-duplicate knots (denominator < 5e-7) are recomputed
exactly on host (~0.5% of curves), as in the reference's own numerics.
"""

import numpy as np
from contextlib import ExitStack

import concourse.bass as bass
import concourse.mybir as mybir
import concourse.tile as tile
from concourse import library_config
from concourse.bass_utils import run_bass_kernel_spmd

# ---------------------------------------------------------------- constants
B, M, PDEG, DIM, T, K = 8192, 100, 3, 3, 512, 104
NCORES = 8
BL = B // NCORES      # curves per core
PT = 128              # curves per tile (partition dim)
NI = 96               # interior knots per curve
NSP = 97              # spans (i = 3..99)
NSEG = 13             # staircases: x-base + 12 coeffs

U0 = np.float32(1e-5)
UEND = np.float32(1.0 - 1e-5)
STEP = np.float32((UEND - U0) / np.float32(511.0))
EPS8 = np.float32(1e-8)
MAGIC = np.float32(12582912.0)   # 1.5*2^23: float round-to-int magic

F32 = mybir.dt.float32
I16 = mybir.dt.int16

AOP = mybir.AluOpType

LS_C1 = np.float32(1.0) / np.float32(511.0)
LS_C2 = np.float32(UEND * (np.float32(1.0) / np.float32(511.0)))


def _u_grid() -> np.ndarray:
    # bitwise replica of jnp.linspace(1e-5, 1-1e-5, 512, float32) as
    # compiled by XLA CPU (verified bit-exact against the jitted fusion).
    t = np.arange(T, dtype=np.float32)
    step = (t * LS_C1).astype(np.float32)
    om = (np.float32(1.0) - step).astype(np.float32)
    u = np.float32(
        np.float64(U0) * np.float64(om) + np.float64(t) * np.float64(LS_C2)
    ).astype(np.float32)
    u[511] = UEND
    return u


# ------------------------------------------------------------- bass program
def _build_nc(nt: int = BL // PT) -> bass.Bass:
    nc = bass.Bass()
    bl = nt * PT
    ctrl = nc.declare_dram_parameter("ctrl", [bl, M * DIM], F32, isOutput=False)
    knots = nc.declare_dram_parameter("knots", [bl, K], F32, isOutput=False)
    uin = nc.declare_dram_parameter("u", [PT, T], F32, isOutput=False)
    out = nc.declare_dram_parameter("out", [bl, T * DIM], F32, isOutput=True)

    with tile.TileContext(nc) as tc, ExitStack() as ctx:
        singles = ctx.enter_context(tc.tile_pool(name="singles", bufs=1))
        io = ctx.enter_context(tc.tile_pool(name="io", bufs=6))
        small = ctx.enter_context(tc.tile_pool(name="small", bufs=3))
        kpool = ctx.enter_context(tc.tile_pool(name="kpool", bufs=2))
        dsts = ctx.enter_context(tc.tile_pool(name="dsts", bufs=20))
        stairs = ctx.enter_context(tc.tile_pool(name="stairs", bufs=25))
        work = ctx.enter_context(tc.tile_pool(name="work", bufs=3))
        outp = ctx.enter_context(tc.tile_pool(name="outp", bufs=3))

        nc.gpsimd.load_library(library_config.local_scatter)
        u_t = singles.tile([PT, T], F32)
        nc.sync.dma_start(out=u_t[:, :], in_=uin[:, :])
        ones16 = singles.tile([PT, 98], I16)
        nc.vector.memset(ones16[:, :], 1)
        neg1 = singles.tile([PT, NI], F32)
        nc.vector.memset(neg1[:, :], -1.0)

        def fe(it):
            r0 = it * PT
            Ud = io.tile([PT, K], F32, tag="Ud")
            nc.sync.dma_start(out=Ud[:, :], in_=knots[r0 : r0 + PT, :])
            # bounce for the seg-0 scatter (single-producer rule)
            U = io.tile([PT, K], F32, tag="U")
            nc.vector.tensor_copy(U[:, :], Ud[:, :])
            Pd = []
            for d in range(DIM):
                pdd = io.tile([PT, M], F32, tag=f"Pd{d}", name=f"pdd{d}_{it}")
                nc.sync.dma_start(
                    out=pdd[:, :], in_=ctrl[r0 : r0 + PT, d :: DIM]
                )
                Pd.append(pdd)

            # ---------------- stage A: span boundaries c_j  [PT, NI] -----
            intr = U[:, 4 : 4 + NI]
            q = small.tile([PT, NI], F32, tag="q")
            nc.vector.tensor_scalar(
                q[:, :], intr,
                float(EPS8) - float(U0), float(1.0 / np.float64(STEP)),
                AOP.add, AOP.mult,
            )
            c0 = small.tile([PT, NI], F32, tag="c0")
            nc.vector.tensor_scalar(
                c0[:, :], q[:, :], float(MAGIC), float(-MAGIC), AOP.add, AOP.add
            )
            acc = small.tile([PT, NI], F32, tag="acc")
            tauu = small.tile([PT, NI], F32, tag="tauu")
            stt = small.tile([PT, NI], F32, tag="stt")
            om = small.tile([PT, NI], F32, tag="om")
            ge = small.tile([PT, NI], F32, tag="ge")
            for i, dlt in enumerate((-2.0, -1.0, 0.0, 1.0)):
                nc.vector.tensor_scalar(
                    tauu[:, :], c0[:, :], dlt, None, AOP.add
                )
                nc.vector.tensor_scalar(
                    stt[:, :], tauu[:, :], float(LS_C1), None, AOP.mult
                )
                nc.vector.tensor_scalar(
                    om[:, :], stt[:, :], -float(U0), float(U0),
                    AOP.mult, AOP.add,
                )
                nc.vector.tensor_scalar(
                    stt[:, :], tauu[:, :], float(LS_C2), None, AOP.mult
                )
                nc.vector.tensor_tensor(tauu[:, :], stt[:, :], om[:, :], AOP.add)
                nc.vector.tensor_tensor(tauu[:, :], tauu[:, :], intr, AOP.subtract)
                dst_g = acc if i == 0 else ge
                nc.vector.tensor_scalar(
                    dst_g[:, :], tauu[:, :], float(EPS8), None, AOP.is_gt
                )
                if i > 0:
                    nc.vector.tensor_tensor(acc[:, :], acc[:, :], ge[:, :], AOP.add)
            cc = small.tile([PT, NI], F32, tag="cc")
            nc.vector.tensor_scalar(cc[:, :], c0[:, :], 2.0, None, AOP.add)
            nc.vector.tensor_tensor(cc[:, :], cc[:, :], acc[:, :], AOP.subtract)
            nc.vector.tensor_scalar(
                cc[:, :], cc[:, :], 0.0, 511.0, AOP.max, AOP.min
            )
            eq = small.tile([PT, NI - 1], mybir.dt.uint8, tag="eq")
            nc.vector.tensor_tensor(
                eq[:, :], cc[:, : NI - 1], cc[:, 1:NI], AOP.is_equal
            )
            nc.vector.copy_predicated(cc[:, : NI - 1], eq[:, :], neg1[:, : NI - 1])

            # cval [PT, NSP]: col0 = init idx (0, or -1 if a boundary bin
            # is 0), cols 1..96 = deduped boundary bins
            cval = small.tile([PT, NSP], F32, tag="cval")
            iszero = small.tile([PT, 1], F32, tag="iszero")
            nc.vector.tensor_scalar(
                iszero[:, :], cc[:, 0:1], 0.0, None, AOP.is_equal
            )
            nc.vector.tensor_scalar(
                cval[:, 0:1], iszero[:, :], -1.0, None, AOP.mult
            )
            nc.vector.tensor_copy(cval[:, 1:NSP], cc[:, :])

            # flag idx [PT, 98] i16 (col 97 = -1 pad)
            fidx = small.tile([PT, 98], I16, tag="fidx")
            nc.vector.memset(fidx[:, :], -1)
            nc.vector.tensor_copy(fidx[:, 0:NSP], cval[:, :])
            # pair idx [PT, 2*NSP] i16
            c2 = small.tile([PT, NSP], F32, tag="c2")
            nc.vector.tensor_scalar(c2[:, :], cval[:, :], 2.0, None, AOP.mult)
            idxp = small.tile([PT, 2 * NSP], I16, tag="idxp")
            idxp_v = idxp[:, :].rearrange("p (a b) -> p a b", b=2)
            nc.vector.tensor_copy(idxp_v[:, :, 0], c2[:, :])
            nc.vector.tensor_scalar(c2[:, :], c2[:, :], 1.0, None, AOP.add)
            nc.vector.tensor_copy(idxp_v[:, :, 1], c2[:, :])

            # ---------------- K coefficients per span [PT, NSP] ----------
            def ks(name):
                return kpool.tile([PT, NSP], F32, tag=name, name=f"{name}_{it}")

            Ui = Ud[:, 3 : 3 + NSP]
            dm2 = ks("dm2"); dm1 = ks("dm1")
            d1 = ks("d1"); d2 = ks("d2"); d3 = ks("d3")
            nc.vector.tensor_tensor(dm2[:, :], Ud[:, 1 : 1 + NSP], Ui, AOP.subtract)
            nc.vector.tensor_tensor(dm1[:, :], Ud[:, 2 : 2 + NSP], Ui, AOP.subtract)
            nc.vector.tensor_tensor(d1[:, :], Ud[:, 4 : 4 + NSP], Ui, AOP.subtract)
            nc.vector.tensor_tensor(d2[:, :], Ud[:, 5 : 5 + NSP], Ui, AOP.subtract)
            nc.vector.tensor_tensor(d3[:, :], Ud[:, 6 : 6 + NSP], Ui, AOP.subtract)
            den = ks("den")
            rscr = ks("rscr")
            R1 = ks("R1"); R20 = ks("R20"); R21 = ks("R21")
            R30 = ks("R30"); R31 = ks("R31"); R32 = ks("R32")
            nc.vector.reciprocal_approx_accurate(R1[:, :], d1[:, :], scratch=rscr[:, :])
            nc.vector.tensor_tensor(den[:, :], d1[:, :], dm1[:, :], AOP.subtract)
            nc.vector.reciprocal_approx_accurate(R20[:, :], den[:, :], scratch=rscr[:, :])
            nc.vector.reciprocal_approx_accurate(R21[:, :], d2[:, :], scratch=rscr[:, :])
            nc.vector.tensor_tensor(den[:, :], d1[:, :], dm2[:, :], AOP.subtract)
            nc.vector.reciprocal_approx_accurate(R30[:, :], den[:, :], scratch=rscr[:, :])
            nc.vector.tensor_tensor(den[:, :], d2[:, :], dm1[:, :], AOP.subtract)
            nc.vector.reciprocal_approx_accurate(R31[:, :], den[:, :], scratch=rscr[:, :])
            nc.vector.reciprocal_approx_accurate(R32[:, :], d3[:, :], scratch=rscr[:, :])
            # level 2 (DVE)
            m1 = ks("m1"); m2 = ks("m2"); A20 = ks("A20")
            B20 = ks("B20"); C20 = ks("C20"); E2 = ks("E2")
            nc.vector.tensor_tensor(m1[:, :], R1[:, :], R20[:, :], AOP.mult)
            nc.vector.tensor_tensor(m2[:, :], R1[:, :], R21[:, :], AOP.mult)
            nc.vector.tensor_tensor(A20[:, :], d1[:, :], R20[:, :], AOP.mult)
            nc.vector.tensor_tensor(B20[:, :], d1[:, :], m1[:, :], AOP.mult)
            nc.vector.tensor_tensor(B20[:, :], B20[:, :], R20[:, :], AOP.add)
            nc.vector.tensor_scalar(C20[:, :], A20[:, :], -1.0, None, AOP.add)
            nc.vector.tensor_tensor(E2[:, :], m1[:, :], m2[:, :], AOP.add)

            # level 3 -> true-signed basis coeffs in wide tiles (DVE)
            # NW_l [PT, 4*NSP]: block c holds coeff of x^c of basis l
            def pt(name):
                return kpool.tile([PT, NSP], F32, tag=name, name=f"{name}_{it}")

            def wt(name):
                return kpool.tile(
                    [PT, 4 * NSP], F32, tag=name, name=f"{name}_{it}"
                )

            def tt(o, a, b, op):
                nc.vector.tensor_tensor(o, a, b, op)

            NW0 = wt("NW0"); NW1 = wt("NW1"); NW2 = wt("NW2")

            def blk(w, c):
                return w[:, c * NSP : (c + 1) * NSP]

            T0 = pt("T0"); T1 = pt("T1"); T2 = pt("T2")
            tt(T0[:, :], A20[:, :], R30[:, :], AOP.mult)
            tt(T1[:, :], B20[:, :], R30[:, :], AOP.mult)
            tt(T2[:, :], m1[:, :], R30[:, :], AOP.mult)
            tmp = pt("tmp")
            # NW0 = (d1 - x) * (T0, -T1, T2)
            tt(blk(NW0, 0), d1[:, :], T0[:, :], AOP.mult)        # p00
            tt(tmp[:, :], d1[:, :], T1[:, :], AOP.mult)
            nc.vector.scalar_tensor_tensor(
                blk(NW0, 1), tmp[:, :], -1.0, T0[:, :], AOP.mult, AOP.subtract
            )                                                    # -(d1T1+T0)
            tt(tmp[:, :], d1[:, :], T2[:, :], AOP.mult)
            tt(blk(NW0, 2), tmp[:, :], T1[:, :], AOP.add)        # p02
            nc.vector.tensor_scalar(
                blk(NW0, 3), T2[:, :], -1.0, None, AOP.mult
            )                                                    # -T2
            # saved3 (true values)
            s0 = pt("s0"); s1 = pt("s1"); s2 = pt("s2")
            tt(s0[:, :], A20[:, :], blk(NW0, 0), AOP.subtract)
            nc.vector.scalar_tensor_tensor(
                s1[:, :], blk(NW0, 1), -1.0, B20[:, :], AOP.mult, AOP.subtract
            )                                                    # p01 - B20
            tt(s2[:, :], m1[:, :], blk(NW0, 2), AOP.subtract)
            # r=1 temps
            Uc0 = pt("Uc0"); Uc1 = pt("Uc1"); Uc2 = pt("Uc2")
            tt(Uc0[:, :], C20[:, :], R31[:, :], AOP.mult)
            tt(Uc1[:, :], B20[:, :], R31[:, :], AOP.mult)
            tt(Uc2[:, :], E2[:, :], R31[:, :], AOP.mult)
            q0 = pt("q0"); q1 = pt("q1"); q2 = pt("q2")
            tt(q0[:, :], d2[:, :], Uc0[:, :], AOP.mult)
            tt(q1[:, :], d2[:, :], Uc1[:, :], AOP.mult)
            tt(q1[:, :], q1[:, :], Uc0[:, :], AOP.add)
            tt(q2[:, :], d2[:, :], Uc2[:, :], AOP.mult)
            tt(q2[:, :], q2[:, :], Uc1[:, :], AOP.add)
            # NW1 = saved3 + (d2 - x) * temp   (all true-signed)
            tt(blk(NW1, 0), s0[:, :], q0[:, :], AOP.subtract)
            tt(blk(NW1, 1), s1[:, :], q1[:, :], AOP.add)
            tt(blk(NW1, 2), s2[:, :], q2[:, :], AOP.subtract)
            tt(blk(NW1, 3), T2[:, :], Uc2[:, :], AOP.add)
            # NW2 = saved3' + (0,0,m2,-W)
            W = pt("W")
            tt(W[:, :], m2[:, :], R32[:, :], AOP.mult)
            tt(blk(NW2, 0), q0[:, :], C20[:, :], AOP.subtract)   # sp0
            tt(blk(NW2, 1), B20[:, :], q1[:, :], AOP.subtract)   # sp1
            tt(tmp[:, :], q2[:, :], E2[:, :], AOP.subtract)      # sp2
            tt(blk(NW2, 2), tmp[:, :], m2[:, :], AOP.add)
            nc.vector.scalar_tensor_tensor(
                blk(NW2, 3), Uc2[:, :], -1.0, W[:, :], AOP.mult, AOP.subtract
            )                                                    # -Uc2 - W

            # wide combine: KW_d = sum_l NW_l * P_l (l=0..2 wide, l=3 c3 fix)
            from concourse.bass import AP as _AP

            def prep4(apx):
                return _AP(apx.tensor, apx.offset, [apx.ap[0], [0, 4], [1, NSP]])

            KW = []
            kwtmp = kpool.tile([PT, 4 * NSP], F32, tag="kwtmp", name=f"kwtmp_{it}")
            w4 = kwtmp[:, :].rearrange("p (r q) -> p r q", r=4)
            for d in range(DIM):
                kw = kpool.tile(
                    [PT, 4 * NSP], F32, tag=f"KW{d}", name=f"KW{d}_{it}"
                )
                KW.append(kw)
                kv = kw[:, :].rearrange("p (r q) -> p r q", r=4)
                nc.vector.tensor_tensor(
                    kv, NW0[:, :].rearrange("p (r q) -> p r q", r=4),
                    prep4(Pd[d][:, 0 : 0 + NSP]), AOP.mult,
                )
                nc.vector.tensor_tensor(
                    w4, NW1[:, :].rearrange("p (r q) -> p r q", r=4),
                    prep4(Pd[d][:, 1 : 1 + NSP]), AOP.mult,
                )
                nc.vector.tensor_tensor(kw[:, :], kw[:, :], kwtmp[:, :], AOP.add)
                nc.vector.tensor_tensor(
                    w4, NW2[:, :].rearrange("p (r q) -> p r q", r=4),
                    prep4(Pd[d][:, 2 : 2 + NSP]), AOP.mult,
                )
                nc.vector.tensor_tensor(kw[:, :], kw[:, :], kwtmp[:, :], AOP.add)
                # l=3 only contributes to the c3 block
                tt(tmp[:, :], W[:, :], Pd[d][:, 3 : 3 + NSP], AOP.mult)
                tt(blk(kw, 3), blk(kw, 3), tmp[:, :], AOP.add)

            # ---------------- scatters (Pool, scatter lib) ---------------
            flagd = dsts.tile([PT, T], I16, tag="flagd", bufs=3)
            nc.gpsimd.local_scatter(
                flagd[:, :], ones16[:, :], fidx[:, :],
                channels=PT, num_elems=T, num_idxs=98,
            )
            segdst = []
            sdata = [U[:, 3 : 3 + NSP]] + [
                KW[d][:, c * NSP : (c + 1) * NSP]
                for c in range(4)
                for d in range(DIM)
            ]
            for s in range(NSEG):
                dst = dsts.tile(
                    [PT, 2 * T], I16, tag="dst", name=f"dst{s}_{it}"
                )
                segdst.append(dst)
                nc.gpsimd.local_scatter(
                    dst[:, :], sdata[s].bitcast(I16),
                    idxp[:, :], channels=PT, num_elems=2 * T, num_idxs=2 * NSP,
                )

            return dict(it=it, flagd=flagd, segdst=segdst, u_t=u_t)

        def be(st):
            it = st["it"]; flagd = st["flagd"]; segdst = st["segdst"]
            r0 = it * PT
            # ---------------- m (ACT) + scans (DVE) ----------------------
            m = work.tile([PT, T], F32, tag="m", bufs=4)
            nc.scalar.activation(
                m[:, :], flagd[:, :], mybir.ActivationFunctionType.Copy,
                bias=1.0, scale=-1.0,
            )
            # fence: DVE reads m so the scans' ACT dep is covered by DVE
            # program order (their TensorScalarPtr 1-wait budget goes to
            # the Pool scatter wait)
            fence = work.tile([PT, 2], F32, tag="fence")
            nc.vector.tensor_tensor(
                fence[:, :], m[:, 0:2], m[:, 0:2], AOP.add
            )
            segst = []
            for s in range(NSEG):
                st = stairs.tile([PT, T], F32, tag="stair", name=f"st{s}_{it}")
                segst.append(st)
                nc.vector.tensor_tensor_scan(
                    st[:, :], m[:, :], segdst[s][:, :].bitcast(F32), m[:, 0:1],
                    AOP.mult, AOP.add,
                )

            # ---------------- Horner (DVE) -------------------------------
            def seg(s):
                return segst[s][:, :]

            x = work.tile([PT, T], F32, tag="x")
            nc.vector.scalar_tensor_tensor(
                x[:, :], seg(0), -1.0, u_t[:, :], AOP.mult, AOP.add
            )
            ob = outp.tile([PT, T * DIM], F32, tag="ob")
            obv = ob[:, :].rearrange("p (t d) -> p t d", d=DIM)
            h = work.tile([PT, T], F32, tag="h")
            for d in range(DIM):
                nc.vector.tensor_tensor(h[:, :], seg(1 + 3 * DIM + d), x[:, :], AOP.mult)
                nc.vector.tensor_tensor(h[:, :], h[:, :], seg(1 + 2 * DIM + d), AOP.add)
                nc.vector.tensor_tensor(h[:, :], h[:, :], x[:, :], AOP.mult)
                nc.vector.tensor_tensor(h[:, :], h[:, :], seg(1 + 1 * DIM + d), AOP.add)
                nc.vector.tensor_tensor(h[:, :], h[:, :], x[:, :], AOP.mult)
                nc.vector.tensor_tensor(
                    obv[:, :, d], h[:, :], seg(1 + 0 * DIM + d), AOP.add
                )

            nc.sync.dma_start(out=out[r0 : r0 + PT, :], in_=ob[:, :])

        prev = None
        for it in range(nt):
            cur = fe(it)
            if prev is not None:
                be(prev)
            prev = cur
        be(prev)

    _post_process(nc)
    return nc


# --------------------------------------------------- post-Tile IR surgery
POOL_STD_TYPES = ("InstTensorTensor", "InstTensorReduce", "InstIota", "InstPool")


def _post_process(nc):
    import bass_rust as _br

    # 1. re-insert pool library reloads at transitions (the Tile scheduler
    #    hoists dep-less reload pseudo-instructions to the block head).
    made = {
        7: [nc.gpsimd.load_library(library_config.local_scatter).ins for _ in range(600)],
        0: [nc.gpsimd.load_library(library_config.standard).ins for _ in range(600)],
    }
    cur = None
    nrel = 0
    for func in nc.m.functions:
        for blk in func.blocks:
            new_insts = []
            for inst in blk.instructions:
                tn = type(inst).__name__
                if tn == "InstPseudoReloadLibraryIndex":
                    continue
                need = None
                if tn == "InstLocalScatter":
                    need = library_config.local_scatter
                elif tn in POOL_STD_TYPES and str(
                    getattr(inst, "engine", "")
                ).endswith("Pool"):
                    need = library_config.standard
                if need is not None and cur != need.index:
                    new_insts.append(made[need.index].pop())
                    cur = need.index
                    nrel += 1
                new_insts.append(inst)
            blk.instructions = new_insts

    # 2. lower extended-inst ISA payloads (LocalScatter, reloads)
    from concourse.library_overlay import lower_extended_insts

    lower_extended_insts(nc)

    # 3. sync-wait budget fixes.
    for inst in nc.all_instructions():
        tn = type(inst).__name__
        si = inst.sync_info
        if not si or len(si.on_wait) <= 1:
            continue
        if tn == "InstLocalScatter":
            # generic ISA encoding: 1 wait. Pool-side deps (K data, slot
            # WAW vs pool ops) are implied by engine program order; keep
            # the DVE wait (idx / U-copy producers).
            keep = [w for w in si.on_wait if "DVE" in w.ant_name][-1:]
            if not keep:
                keep = list(si.on_wait)[:1]
            inst.sync_info = _br.SyncInfo(on_wait=keep, on_update=si.on_update)
        elif tn == "InstActivation":
            keep = [w for w in si.on_wait if "Pool" in w.ant_name][-1:] or list(
                si.on_wait
            )[:1]
            inst.sync_info = _br.SyncInfo(on_wait=keep, on_update=si.on_update)
        elif tn == "InstTensorScalarPtr":
            # scans: keep the Pool wait (scatter producer); DVE-side deps
            # (m via fence) are program-order implied.
            keep = [w for w in si.on_wait if "Pool" in w.ant_name][-1:]
            if not keep:
                keep = [w for w in si.on_wait if "DVE" not in w.ant_name][:1]
            if not keep:
                keep = list(si.on_wait)[:1]
            inst.sync_info = _br.SyncInfo(on_wait=keep, on_update=si.on_update)
        elif tn == "InstTensorTensor" and str(
            getattr(inst, "engine", "")
        ).endswith("DVE"):
            # DVE self-waits are implied by engine program order; a DMAHW
            # wait uses two encoding slots, so 2 raw waits over-budget.
            keep = [w for w in si.on_wait if "DVE" not in w.ant_name]
            if keep and len(keep) < len(si.on_wait):
                inst.sync_info = _br.SyncInfo(
                    on_wait=keep, on_update=si.on_update
                )
        elif tn == "InstDMACopy":
            keep = [w for w in si.on_wait if "DMAHW" not in w.ant_name]
            if len(keep) == 1:
                inst.sync_info = _br.SyncInfo(
                    on_wait=keep, on_update=si.on_update
                )

    # 4. kernel-tail drain wait spreading (as v1)
    insts = list(nc.all_instructions())
    big_i = None
    for i, inst in enumerate(insts):
        si = inst.sync_info
        if type(inst).__name__ == "InstDrain" and si and len(si.on_wait) > 2:
            big_i = i
            break
    if big_i is not None:
        last_q = {}
        for inst in insts[:big_i]:
            if type(inst).__name__ == "InstDMACopy" and inst.sync_info:
                is_out = any(
                    "out" in str(getattr(o, "memref", "")) for o in inst.outs
                )
                for u in inst.sync_info.on_update:
                    if "DMAHW" in u.ant_name:
                        last_q[u.ant_name] = is_out
        drain = insts[big_i]
        req = [
            w
            for w in drain.sync_info.on_wait
            if "DMAHW" in w.ant_name and last_q.get(w.ant_name, True)
        ]
        assert req, drain.sync_info.on_wait
        drain.sync_info = _br.SyncInfo(
            on_wait=req[:1], on_update=drain.sync_info.on_update
        )
        todo = req[1:]
        for inst in insts[big_i - 8 :]:
            if not todo:
                break
            if inst is drain:
                continue
            si = inst.sync_info
            if type(inst).__name__ in (
                "InstDrain",
                "InstEventSemaphore",
                "InstUnconditionalBranch",
            ) and (not si or not si.on_wait):
                inst.sync_info = _br.SyncInfo(
                    on_wait=[todo.pop(0)],
                    on_update=(si.on_update if si else []),
                )
        assert not todo, f"unplaced drain waits: {todo}"


_NC_CACHE: list = [None]
TRACE = False
LAST_RESULTS: list = [None]


def _get_nc():
    if _NC_CACHE[0] is None:
        _NC_CACHE[0] = _build_nc()
    return _NC_CACHE[0]


# ------------------------------------------------------- host-side helpers
def _ref_numpy(ctrl_pts: np.ndarray, knot_u: np.ndarray) -> np.ndarray:
    """Exact f32 replica of the jax reference for a subset of curves."""
    n = ctrl_pts.shape[0]
    u = _u_grid()
    Uk = knot_u
    diff = u[None, None, :] - Uk[:, PDEG:-PDEG, None]
    masked = np.where(diff > EPS8, diff, np.float32(1.0))
    uspan = np.argmin(masked, axis=1).astype(np.int64) + PDEG

    def gknots(off):
        return np.take_along_axis(Uk, uspan + off, axis=1)

    Ni = [None] * (PDEG + 1)
    Ni[0] = np.broadcast_to(np.ones_like(u), (n, T)).copy()
    for k in range(1, PDEG + 1):
        saved = np.zeros((n, T), np.float32)
        for r in range(k):
            U1 = gknots(r + 1)
            U2 = gknots(1 - k + r)
            denom = (U1 - u[None, :]) + (u[None, :] - U2)
            safe = np.where(denom == 0.0, np.float32(1.0), denom)
            temp = np.where(denom == 0.0, np.float32(1e-4), Ni[r] / safe)
            Ni[r] = saved + (U1 - u[None, :]) * temp
            saved = (u[None, :] - U2) * temp
        Ni[k] = saved
    Nu = np.stack(Ni, axis=1)
    idx = uspan[:, :, None] - PDEG + np.arange(PDEG + 1)
    pts = ctrl_pts[np.arange(n)[:, None, None], idx]
    curve = np.einsum("blt,btld->btd", Nu, pts).astype(np.float32)
    return curve


def _flag_curves(knot_u: np.ndarray) -> np.ndarray:
    """Curves where some Cox-de-Boor denominator U[i+k]-U[i] is < 5e-7."""
    bad = np.zeros(knot_u.shape[0], dtype=bool)
    for k, ilo in ((1, 3), (2, 2), (3, 1)):
        g = knot_u[:, ilo + k : 100 + k] - knot_u[:, ilo:100]
        bad |= (g < np.float32(5e-7)).any(axis=1)
    return bad


# ---------------------------------------------------------------- entry
def kernel(ctrl_pts: np.ndarray, knot_u: np.ndarray) -> np.ndarray:
    ctrl_pts = np.ascontiguousarray(ctrl_pts, dtype=np.float32)
    knot_u = np.ascontiguousarray(knot_u, dtype=np.float32)

    nc = _get_nc()
    u_rep = np.broadcast_to(_u_grid()[None, :], (PT, T)).copy()

    in_maps = []
    for c in range(NCORES):
        sl = slice(c * BL, (c + 1) * BL)
        in_maps.append(
            {
                "ctrl": ctrl_pts[sl].reshape(BL, M * DIM),
                "knots": knot_u[sl],
                "u": u_rep,
            }
        )
    res = run_bass_kernel_spmd(
        nc, in_maps, core_ids=list(range(NCORES)), trace=TRACE
    )
    LAST_RESULTS[0] = res
    out = np.concatenate(
        [res.results[c]["out"].reshape(BL, T, DIM) for c in range(NCORES)], axis=0
    )

    bad = _flag_curves(knot_u)
    if bad.any():
        out[bad] = _ref_numpy(ctrl_pts[bad], knot_u[bad])
    return out
